# revision 24
# baseline (speedup 1.0000x reference)
"""Trainium2 Bass kernel: masked softmax attention (B=2, H=16, S=2048, Dk=64).

Returns (context, attn) like the reference:
    scores = Q K^T / sqrt(dk); scores[mask] = -1e9; attn = softmax(scores);
    context = attn @ V

Sharding: B*H = 32 heads -> 8 NeuronCores x 4 heads (pure data parallel).

Device algorithm (per head), all in "transposed" space so that the P@V
matmul needs no on-chip transposes (contraction dim lands on partitions):
  - Host pre-transposes Q, K and the mask; host re-transposes outputs.
  - S^T[k,q] = K Q^T via PE (f32r, 1 cyc/row), one 128-row k-tile at a time.
  - The mask is folded in on the PE: an identity-stationary matmul
    accumulates maskneg^T (fp8 e5m2 values {0, -16384}) into the scores
    PSUM; exp then underflows masked entries to exactly 0.0.
  - E^T = exp(S^T/8) on ACT (PSUM->SBUF, fused scale).
  - ctx^T[d,q] accumulates V_aug[k,d+1]^T @ E^T over k-tiles, where V_aug
    carries a ones column => row 64 of ctx^T is the softmax denominator.
  - recip = 1/sums on DVE; broadcast across partitions via a rank-1 PE
    matmul (ones x recip); P^T = E^T * bcast on DVE; ctx rows scaled too.
"""

import numpy as np

B, H, S, DK = 2, 16, 2048, 64
N_CORES = 8
HPC = (B * H) // N_CORES  # heads per core
NKT = S // 128  # 16 k-tiles per head
NQB = S // 512  # 4 q-blocks of 512
SCALE = 0.125  # 1/sqrt(64)
MASK_NEG = -16384.0  # exp(0.125*(s-16384)) underflows to +0.0 in fp32
E5M2_NEG16384 = 0xF4  # fp8e5m2 bit pattern of -16384.0

_cache = {}


def _build_nc():
    """Build + compile the per-core Bass program (same program on all cores)."""
    import os
    from contextlib import ExitStack

    STORE_ENG = os.environ.get("K_STORE_ENG", "gpsimd")  # gpsimd | sync
    BCAST = os.environ.get("K_BCAST", "dma")  # dma | pe
    PT_DT = os.environ.get("K_PT_DT", "fp16")  # f32 | bf16 | fp16

    import concourse.tile as tile
    from concourse import bacc, mybir

    F32 = mybir.dt.float32
    F32R = mybir.dt.float32r
    F8 = mybir.dt.float8e5
    Exp = mybir.ActivationFunctionType.Exp

    nc = bacc.Bacc("TRN2", target_bir_lowering=False, debug=False)

    qt = nc.dram_tensor("qt", [HPC, DK, S], F32R, kind="ExternalInput")
    kt = nc.dram_tensor("kt", [HPC, DK, S], F32R, kind="ExternalInput")
    v = nc.dram_tensor("v", [HPC, S, DK], F32R, kind="ExternalInput")
    mk = nc.dram_tensor("mk", [HPC, S, S], F8, kind="ExternalInput")
    ident = nc.dram_tensor("ident", [128, 128], F8, kind="ExternalInput")
    pt_dt = {"f32": F32R, "bf16": mybir.dt.bfloat16, "fp16": mybir.dt.float16}[PT_DT]
    pt = nc.dram_tensor("pt", [HPC, S, S], pt_dt, kind="ExternalOutput")
    ct = nc.dram_tensor("ct", [HPC, DK + 1, S], F32, kind="ExternalOutput")

    QW = 1024  # q-block width processed end-to-end (2 blocks per head)

    with tile.TileContext(nc) as tc, ExitStack() as ctx:
        io = ctx.enter_context(tc.tile_pool(name="io", bufs=2))
        ets = ctx.enter_context(tc.tile_pool(name="ets", bufs=2 * NKT - 2))
        mks = ctx.enter_context(tc.tile_pool(name="mks", bufs=4))
        bcp = ctx.enter_context(tc.tile_pool(name="bcp", bufs=2))
        ctxs = ctx.enter_context(tc.tile_pool(name="ctxs", bufs=1))
        small = ctx.enter_context(tc.tile_pool(name="small", bufs=1))
        ones_p = ctx.enter_context(tc.tile_pool(name="ones", bufs=1))
        sps = ctx.enter_context(tc.tile_pool(name="sps", bufs=2, space="PSUM"))
        cps = ctx.enter_context(tc.tile_pool(name="cps", bufs=2, space="PSUM"))
        dscr = ctx.enter_context(tc.tile_pool(name="dscr", bufs=2, space="DRAM"))

        id_sb = ones_p.tile([128, 128], F8)
        nc.sync.dma_start(out=id_sb, in_=ident.ap())
        store = getattr(nc, STORE_ENG)
        ones_col = ones_p.tile([1, 128], F32)
        nc.vector.memset(ones_col, 1.0)

        for h in range(HPC):
            qt_sb = io.tile([DK, S], F32R, tag="qt")
            nc.sync.dma_start(out=qt_sb, in_=qt.ap()[h])
            kt_sb = io.tile([DK, S], F32R, tag="kt")
            nc.sync.dma_start(out=kt_sb, in_=kt.ap()[h])
            vaug = io.tile([128, NKT, DK + 1], F32R, tag="vaug")
            nc.sync.dma_start(
                out=vaug[:, :, 0:DK],
                in_=v.ap()[h].rearrange("(t p) d -> p t d", p=128),
            )
            nc.vector.memset(vaug[:, :, DK : DK + 1].bitcast(F32), 1.0)

            # taper the last head's final blocks so the un-overlappable
            # end-of-kernel normalize+store drain is short
            widths = [QW] * (S // QW)
            if h == HPC - 1 and os.environ.get("K_TAPER"):
                widths = [int(x) for x in os.environ["K_TAPER"].split(",")]
                assert sum(widths) == S
            q0 = 0
            for jb, qw in enumerate(widths):
                ctx_ps = cps.tile([DK + 1, qw], F32, tag="cps")
                et_tiles = []
                for ikt in range(NKT):
                    ks = slice(ikt * 128, (ikt + 1) * 128)
                    et = ets.tile([128, qw], F32R)
                    et_tiles.append(et)
                    mk_sb = mks.tile([128, qw], F8)
                    nc.sync.dma_start(
                        out=mk_sb, in_=mk.ap()[h, ks, q0 : q0 + qw]
                    )
                    s_ps = sps.tile([128, qw], F32, tag="sp")
                    for jq in range(max(1, qw // 512)):
                        ps = slice(jq * 512, min((jq + 1) * 512, qw))
                        nc.tensor.matmul(
                            s_ps[:, ps],
                            kt_sb[:, ks],
                            qt_sb[:, q0 + jq * 512 : q0 + min((jq + 1) * 512, qw)],
                            start=True,
                            stop=False,
                        )
                        nc.tensor.matmul(
                            s_ps[:, ps],
                            id_sb[:, :],
                            mk_sb[:, ps],
                            start=False,
                            stop=True,
                        )
                    nc.scalar.activation(
                        out=et[:, :], in_=s_ps[:, :], func=Exp, scale=SCALE
                    )
                    for jq in range(max(1, qw // 512)):
                        ps = slice(jq * 512, min((jq + 1) * 512, qw))
                        nc.tensor.matmul(
                            ctx_ps[:, ps],
                            vaug[:, ikt, :],
                            et[:, ps],
                            start=(ikt == 0),
                            stop=(ikt == NKT - 1),
                        )

                # denominators -> reciprocal -> partition-broadcast (via DMA
                # through a DRAM bounce; keeps the PE stream stall-free)
                sums_row = small.tile([1, QW], F32, tag="sums", name="sums_row")[:, :qw]
                nc.scalar.copy(sums_row[:, :], ctx_ps[DK : DK + 1, :])
                sums_sq = small.tile([128, QW // 128], F32, tag="sumsq", name="sums_sq")[:, : qw // 128]
                nc.scalar.dma_start(out=sums_sq[:, :], in_=sums_row[:, :])
                recip_sq = small.tile([128, QW // 128], F32, tag="recsq", name="recip_sq")[:, : qw // 128]
                nc.vector.reciprocal(recip_sq, sums_sq)
                bc_sb = bcp.tile([128, QW], F32, tag="bc", name="bc_sb")[:, :qw]
                if BCAST == "dma":
                    rdr = dscr.tile([128, qw // 128], F32, tag="rdr", name="rdr")
                    nc.scalar.dma_start(out=rdr[:, :], in_=recip_sq[:, :])
                    nc.scalar.dma_start(
                        out=bc_sb,
                        in_=rdr.rearrange("p f -> (p f)").partition_broadcast(128),
                    )
                else:
                    recip_row = small.tile([1, QW], F32, tag="recrow", name="recip_row")[:, :qw]
                    nc.scalar.dma_start(out=recip_row[:, :], in_=recip_sq[:, :])
                    bc_ps = sps.tile([128, qw], F32, tag="sp")
                    for jq in range(max(1, qw // 512)):
                        nc.tensor.matmul(
                            bc_ps[:, jq * 512 : (jq + 1) * 512],
                            ones_col[:, :],
                            recip_row[:, jq * 512 : (jq + 1) * 512],
                            start=True,
                            stop=True,
                        )
                    nc.scalar.copy(bc_sb[:, :], bc_ps[:, :])

                # ctx^T: copy out of PSUM, scale rows 0..63 by recip
                ctx_sb = ctxs.tile([DK + 1, qw], F32)
                nc.scalar.copy(ctx_sb[:, :], ctx_ps[:, :])
                nc.vector.tensor_mul(ctx_sb[0:DK, :], ctx_sb[0:DK, :], bc_sb[0:DK, :])
                store.dma_start(out=ct.ap()[h, :, q0 : q0 + qw], in_=ctx_sb)

                # P^T = E^T * bcast (in place), then store
                for ikt in range(NKT):
                    et = et_tiles[ikt]
                    nc.vector.tensor_mul(
                        et[:, :], et[:, :], bc_sb[:, :].bitcast(F32R)
                    )
                    store.dma_start(
                        out=pt.ap()[h, ikt * 128 : (ikt + 1) * 128, q0 : q0 + qw],
                        in_=et[:, :].bitcast(F32) if PT_DT != "f32" else et,
                    )
                q0 += qw

    nc.compile()
    return nc


def _prep_in_maps(Q, K, V, attention_mask):
    import ml_dtypes

    Q = np.asarray(Q, dtype=np.float32).reshape(B * H, S, DK)
    K = np.asarray(K, dtype=np.float32).reshape(B * H, S, DK)
    V = np.asarray(V, dtype=np.float32).reshape(B * H, S, DK)
    mask = np.asarray(attention_mask).reshape(B * H, S, S)

    QT = np.ascontiguousarray(Q.transpose(0, 2, 1))  # [BH, DK, S]
    KT = np.ascontiguousarray(K.transpose(0, 2, 1))
    Vc = np.ascontiguousarray(V)
    # maskneg^T as fp8 e5m2 bytes: masked -> -16384.0 (0xF4), keep -> 0.0
    MKNEG = (
        np.ascontiguousarray(mask.transpose(0, 2, 1)).astype(np.uint8) * E5M2_NEG16384
    )
    # fp8e5m2: 1.0 = sign0 exp01111 mant00 -> 0b00111100 = 0x3C
    ident = np.where(np.eye(128, dtype=bool), np.uint8(0x3C), np.uint8(0))

    f8 = ml_dtypes.float8_e5m2
    in_maps = []
    for c in range(N_CORES):
        s = slice(c * HPC, (c + 1) * HPC)
        in_maps.append(
            {
                "qt": QT[s],
                "kt": KT[s],
                "v": Vc[s],
                "mk": MKNEG[s].view(f8),
                "ident": ident.view(f8),
            }
        )
    return in_maps


def _postprocess(results):
    attn = np.empty((B * H, S, S), dtype=np.float32)
    context = np.empty((B * H, S, DK), dtype=np.float32)
    for c in range(N_CORES):
        s = slice(c * HPC, (c + 1) * HPC)
        attn[s] = results[c]["pt"].transpose(0, 2, 1)
        context[s] = results[c]["ct"][:, :DK, :].transpose(0, 2, 1)
    return (
        context.reshape(B, H, S, DK),
        attn.reshape(B, H, S, S),
    )


def kernel(Q, K, V, attention_mask):
    from concourse.bass_utils import run_bass_kernel_spmd

    in_maps = _prep_in_maps(Q, K, V, attention_mask)
    if "nc" not in _cache:
        _cache["nc"] = _build_nc()
    nc = _cache["nc"]

    res = run_bass_kernel_spmd(nc, in_maps, core_ids=list(range(N_CORES)))
    _cache["last_res"] = res  # lets a test harness read exec_time_ns / trace

    return _postprocess(res.results)


# revision 34
# speedup vs baseline: 73669.6868x; 73669.6868x over previous
"""Trainium2 Bass kernel: masked softmax attention (B=2, H=16, S=2048, Dk=64).

Returns (context, attn) like the reference:
    scores = Q K^T / sqrt(dk); scores[mask] = -1e9; attn = softmax(scores);
    context = attn @ V

Sharding: B*H = 32 heads -> 8 NeuronCores x 4 heads each (data parallel,
no cross-core communication).

Device algorithm (per head, processed in two 1024-wide q-blocks), all in
"transposed" space so the P@V matmul needs no on-chip transposes (the
contraction dim lands on partitions):
  - Host pre-transposes Q, K and the mask; host re-transposes outputs.
  - S^T[k,q] = K Q^T on the PE (f32r operands stream 1 cycle/row), one
    128-row k-tile at a time into PSUM.
  - The mask is folded in on the PE: an identity-stationary matmul
    accumulates maskneg^T (fp8 e5m2 values {0, -16384}) into the scores
    PSUM; exp then underflows masked entries to exactly +0.0 (matching the
    reference's exact zeros). Unmasked entries add literal 0.0 (exact).
  - E^T = exp(S^T/8) on ACT (PSUM->SBUF, fused 1/sqrt(dk) scale). The max
    subtraction of a standard softmax is skipped: scores are ~N(0,1) so
    exp cannot overflow, and exp(x-m)/sum(exp(x-m)) == exp(x)/sum(exp(x)).
  - ctx^T[d,q] accumulates V_aug^T @ E^T over k-tiles on the PE, where
    V_aug carries a ones column => row 64 of ctx^T is the softmax
    denominator (no separate reduction pass).
  - recip = 1/sums on DVE (after a DMA reshape to use all 128 lanes);
    broadcast back across partitions with a stride-0 DMA read through a
    DRAM bounce (keeps the PE instruction stream free of the latency
    chain); P^T = E^T * bcast on DVE with an fp16-converting write
    (fp16 keeps all of f32r's effective matmul precision at half the
    store bytes); ctx rows are scaled by the same broadcast.
  - Stores are issued from GPSIMD (SWDGE) and loads from SP (HWDGE) so
    stores never head-of-line-block the next block's mask loads.
"""

import numpy as np

B, H, S, DK = 2, 16, 2048, 64
N_CORES = 8
HPC = (B * H) // N_CORES  # heads per core
NKT = S // 128  # k-tiles per head
QW = 1024  # q-block width processed end-to-end
SCALE = 0.125  # 1/sqrt(64)
E5M2_NEG16384 = 0xF4  # fp8e5m2 bit pattern of -16384.0
E5M2_ONE = 0x3C  # fp8e5m2 bit pattern of 1.0

_cache = {}


def _build_nc():
    """Build + compile the per-core Bass program (same program on all cores)."""
    import os
    from contextlib import ExitStack

    import concourse.bass as bass
    import concourse.tile as tile
    from concourse import bacc, mybir

    # tuning knobs (defaults = the validated/fastest configuration)
    STORE_ENG = os.environ.get("K_STORE_ENG", "gpsimd")  # gpsimd | sync
    BCAST = os.environ.get("K_BCAST", "dma")  # dma | pe
    PT_DT = os.environ.get("K_PT_DT", "fp16")  # f32 | bf16 | fp16
    ETS_BUFS = int(os.environ.get("K_ETS", str(2 * NKT - 6)))

    F32 = mybir.dt.float32
    F32R = mybir.dt.float32r
    F8 = mybir.dt.float8e5
    Exp = mybir.ActivationFunctionType.Exp

    nc = bacc.Bacc("TRN2", target_bir_lowering=False, debug=False)

    qt = nc.dram_tensor("qt", [HPC, DK, S], F32R, kind="ExternalInput")
    kt = nc.dram_tensor("kt", [HPC, DK, S], F32R, kind="ExternalInput")
    v = nc.dram_tensor("v", [HPC, S, DK], F32R, kind="ExternalInput")
    mk = nc.dram_tensor("mk", [HPC, S, S], F8, kind="ExternalInput")
    ident = nc.dram_tensor("ident", [128, 128], F8, kind="ExternalInput")
    pt_dt = {"f32": F32R, "bf16": mybir.dt.bfloat16, "fp16": mybir.dt.float16}[PT_DT]
    pt = nc.dram_tensor("pt", [HPC, S, S], pt_dt, kind="ExternalOutput")
    ct = nc.dram_tensor("ct", [HPC, DK + 1, S], F32, kind="ExternalOutput")

    with tile.TileContext(nc) as tc, ExitStack() as ctx:
        io = ctx.enter_context(tc.tile_pool(name="io", bufs=2))
        ets = ctx.enter_context(tc.tile_pool(name="ets", bufs=ETS_BUFS))
        etf = ctx.enter_context(tc.tile_pool(name="etf", bufs=8))
        mks = ctx.enter_context(tc.tile_pool(name="mks", bufs=4))
        bcp = ctx.enter_context(tc.tile_pool(name="bcp", bufs=2))
        ctxs = ctx.enter_context(tc.tile_pool(name="ctxs", bufs=1))
        small = ctx.enter_context(tc.tile_pool(name="small", bufs=1))
        ones_p = ctx.enter_context(tc.tile_pool(name="ones", bufs=1))
        sps = ctx.enter_context(tc.tile_pool(name="sps", bufs=2, space="PSUM"))
        cps = ctx.enter_context(tc.tile_pool(name="cps", bufs=2, space="PSUM"))
        dscr = ctx.enter_context(tc.tile_pool(name="dscr", bufs=2, space="DRAM"))

        store = getattr(nc, STORE_ENG)
        id_sb = ones_p.tile([128, 128], F8)
        nc.sync.dma_start(out=id_sb, in_=ident.ap())
        ones_col = ones_p.tile([1, 128], F32)
        nc.vector.memset(ones_col, 1.0)

        for h in range(HPC):
            qt_sb = io.tile([DK, S], F32R, tag="qt")
            nc.sync.dma_start(out=qt_sb, in_=qt.ap()[h])
            kt_sb = io.tile([DK, S], F32R, tag="kt")
            nc.sync.dma_start(out=kt_sb, in_=kt.ap()[h])
            vaug = io.tile([128, NKT, DK + 1], F32R, tag="vaug")
            nc.sync.dma_start(
                out=vaug[:, :, 0:DK],
                in_=v.ap()[h].rearrange("(t p) d -> p t d", p=128),
            )
            nc.vector.memset(vaug[:, :, DK : DK + 1].bitcast(F32), 1.0)

            for jb in range(S // QW):  # q-block
                q0 = jb * QW
                ctx_ps = cps.tile([DK + 1, QW], F32, tag="cps")
                et_tiles = []
                for ikt in range(NKT):
                    ks = slice(ikt * 128, (ikt + 1) * 128)
                    et = ets.tile([128, QW], F32R, tag="et", name="et")
                    et_tiles.append(et)
                    mk_sb = mks.tile([128, QW], F8)
                    nc.sync.dma_start(out=mk_sb, in_=mk.ap()[h, ks, q0 : q0 + QW])
                    s_ps = sps.tile([128, QW], F32, tag="sp")
                    for jq in range(QW // 512):
                        ps = slice(jq * 512, (jq + 1) * 512)
                        nc.tensor.matmul(
                            s_ps[:, ps],
                            kt_sb[:, ks],
                            qt_sb[:, q0 + jq * 512 : q0 + (jq + 1) * 512],
                            start=True,
                            stop=False,
                        )
                        nc.tensor.matmul(
                            s_ps[:, ps],
                            id_sb[:, :],
                            mk_sb[:, ps],
                            start=False,
                            stop=True,
                        )
                    nc.scalar.activation(
                        out=et[:, :], in_=s_ps[:, :], func=Exp, scale=SCALE
                    )
                    for jq in range(QW // 512):
                        ps = slice(jq * 512, (jq + 1) * 512)
                        nc.tensor.matmul(
                            ctx_ps[:, ps],
                            vaug[:, ikt, :],
                            et[:, ps],
                            start=(ikt == 0),
                            stop=(ikt == NKT - 1),
                        )

                # denominators -> reciprocal -> partition-broadcast
                sums_row = small.tile([1, QW], F32, tag="sums")
                nc.scalar.copy(sums_row[:, :], ctx_ps[DK : DK + 1, :])
                sums_sq = small.tile([128, QW // 128], F32, tag="sumsq")
                nc.scalar.dma_start(out=sums_sq[:, :], in_=sums_row[:, :])
                recip_sq = small.tile([128, QW // 128], F32, tag="recsq")
                nc.vector.reciprocal(recip_sq, sums_sq)
                bc_sb = bcp.tile([128, QW], F32, tag="bc")
                if BCAST == "dma":
                    # broadcast across partitions with a stride-0 DMA read
                    # through a DRAM bounce (off the PE instruction stream)
                    rdr = dscr.tile([128, QW // 128], F32, tag="rdr")
                    nc.scalar.dma_start(out=rdr[:, :], in_=recip_sq[:, :])
                    nc.scalar.dma_start(
                        out=bc_sb,
                        in_=rdr.rearrange("p f -> (p f)").partition_broadcast(128),
                    )
                else:
                    recip_row = small.tile([1, QW], F32, tag="recrow")
                    nc.scalar.dma_start(out=recip_row[:, :], in_=recip_sq[:, :])
                    bc_ps = sps.tile([128, QW], F32, tag="sp")
                    for jq in range(QW // 512):
                        nc.tensor.matmul(
                            bc_ps[:, jq * 512 : (jq + 1) * 512],
                            ones_col[:, :],
                            recip_row[:, jq * 512 : (jq + 1) * 512],
                            start=True,
                            stop=True,
                        )
                    nc.scalar.copy(bc_sb[:, :], bc_ps[:, :])

                # ctx^T: copy out of PSUM (frees the accumulator slot early),
                # then scale rows 0..63 by recip
                ctx_sb = ctxs.tile([DK + 1, QW], F32)
                nc.scalar.copy(ctx_sb[:, :], ctx_ps[:, :])
                nc.vector.tensor_mul(ctx_sb[0:DK, :], ctx_sb[0:DK, :], bc_sb[0:DK, :])
                store.dma_start(out=ct.ap()[h, :, q0 : q0 + QW], in_=ctx_sb)

                # P^T = E^T * bcast, then store
                for ikt in range(NKT):
                    et = et_tiles[ikt]
                    out_slice = pt.ap()[h, ikt * 128 : (ikt + 1) * 128, q0 : q0 + QW]
                    if PT_DT == "f32":
                        nc.vector.tensor_mul(
                            et[:, :], et[:, :], bc_sb[:, :].bitcast(F32R)
                        )
                        store.dma_start(out=out_slice, in_=et)
                    else:
                        # dtype-converting DVE write; the store needs no cast
                        ef = etf.tile([128, QW], pt_dt, tag="etf", name="ef")
                        nc.vector.tensor_mul(
                            ef[:, :], et[:, :], bc_sb[:, :].bitcast(F32R)
                        )
                        store.dma_start(out=out_slice, in_=ef)

    nc.compile()
    return nc


def _prep_in_maps(Q, K, V, attention_mask):
    import ml_dtypes

    Q = np.asarray(Q, dtype=np.float32).reshape(B * H, S, DK)
    K = np.asarray(K, dtype=np.float32).reshape(B * H, S, DK)
    V = np.asarray(V, dtype=np.float32).reshape(B * H, S, DK)
    mask = np.asarray(attention_mask).reshape(B * H, S, S)

    QT = np.ascontiguousarray(Q.transpose(0, 2, 1))  # [BH, DK, S]
    KT = np.ascontiguousarray(K.transpose(0, 2, 1))
    Vc = np.ascontiguousarray(V)
    # maskneg^T as fp8 e5m2 bytes: masked -> -16384.0, keep -> 0.0
    MKNEG = (
        np.ascontiguousarray(mask.transpose(0, 2, 1)).astype(np.uint8) * E5M2_NEG16384
    )
    ident = np.where(np.eye(128, dtype=bool), np.uint8(E5M2_ONE), np.uint8(0))

    f8 = ml_dtypes.float8_e5m2
    in_maps = []
    for c in range(N_CORES):
        s = slice(c * HPC, (c + 1) * HPC)
        in_maps.append(
            {
                "qt": QT[s],
                "kt": KT[s],
                "v": Vc[s],
                "mk": MKNEG[s].view(f8),
                "ident": ident.view(f8),
            }
        )
    return in_maps


def _postprocess(results):
    attn = np.empty((B * H, S, S), dtype=np.float32)
    context = np.empty((B * H, S, DK), dtype=np.float32)
    for c in range(N_CORES):
        s = slice(c * HPC, (c + 1) * HPC)
        attn[s] = results[c]["pt"].transpose(0, 2, 1)
        context[s] = results[c]["ct"][:, :DK, :].transpose(0, 2, 1)
    return (
        context.reshape(B, H, S, DK),
        attn.reshape(B, H, S, S),
    )


def kernel(Q, K, V, attention_mask):
    from concourse.bass_utils import run_bass_kernel_spmd

    in_maps = _prep_in_maps(Q, K, V, attention_mask)
    if "nc" not in _cache:
        _cache["nc"] = _build_nc()
    nc = _cache["nc"]

    res = run_bass_kernel_spmd(nc, in_maps, core_ids=list(range(N_CORES)))
    _cache["last_res"] = res  # lets a test harness read exec_time_ns / trace

    return _postprocess(res.results)
